# revision 1
# baseline (speedup 1.0000x reference)
"""TRN2 Bass kernel for CP-decoding line-sampling (nn_CPDecoding).

kernel(in_tensor [2097152,3] f32, line_coef [3,24,256] f32) -> [2097152] f32

Math per point n (reference semantics, align_corners grid_sample on R=256):
  pos_d = ((coord_d + 1) * 0.5) * 255          d=0,1,2 over (x,y,z) columns
  i0_d  = floor(pos_d); w_d = pos_d - i0_d
  f_d   = T_d[:, i0] + w_d * (T_d[:, i0+1] - T_d[:, i0])   (T_d = line_coef[2-d])
  out_n = sum_c f_0[c] * f_1[c] * f_2[c]

Strategy: data-parallel over points across 8 NeuronCores. Per core, SWDGE
dma_gather fetches one 256B pair-row (base row ++ delta row, 24->32 f32
padded) per (point, dim) from an HBM table [768, 64]; DVE computes
floor/frac (cast + is_gt fixup -- correct whether the f32->i16 cast rounds
or truncates), the interpolation, 3-way product and component-sum reduce.
The gather's wrapped+replicated index layout is produced by writing the
block-layout indices to a DRAM scratch and reading them back with a
permuted, partition-replicated access pattern. Gathers are split into
1024-index chunks (the SWDGE ring rejects larger instructions here).
"""

import sys

try:
    import concourse.bass  # noqa: F401
except Exception:
    sys.path.insert(0, "/opt/trn_rl_repo")

import numpy as np

import concourse.bacc as bacc
import concourse.bass as bass
import concourse.mybir as mybir
import concourse.tile as tile

F32 = mybir.dt.float32
I16 = mybir.dt.int16
COPY = mybir.ActivationFunctionType.Copy
ALU = mybir.AluOpType

N_TOTAL = 2097152
N_CORES = 8
N_PER_CORE = N_TOTAL // N_CORES
R = 256
C = 24
CP = 32          # padded component stride
ES = 2 * CP      # gather elem_size (64 f32 = 256B)
NT = 8192        # points per tile


def build_ptab(line_coef: np.ndarray) -> np.ndarray:
    """[3, 24, 256] f32 -> [768, 64] pair table (base ++ delta, padded)."""
    line_coef = np.ascontiguousarray(line_coef, dtype=np.float32)
    assert line_coef.shape == (3, C, R)
    pt = np.zeros((3, R, ES), np.float32)
    for b in range(3):
        L = line_coef[2 - b]                      # [24, 256]
        pt[b, :, 0:C] = L.T
        pt[b, 0 : R - 1, CP : CP + C] = (L[:, 1:R] - L[:, 0 : R - 1]).T
    return pt.reshape(3 * R, ES)


def build_kernel(n_per_core: int = N_PER_CORE, nt: int = NT, bufs: int = 2,
                 gchunk: int = 1024):
    assert n_per_core % nt == 0 and nt % 2048 == 0
    assert nt % gchunk == 0 and gchunk % 128 == 0
    tiles = n_per_core // nt
    nch = nt // 128       # chunks (points per partition)
    jw = nt // 16         # wrapped idx columns
    gsub = nt // gchunk   # sub-gathers per dim
    gnch = gchunk // 128  # point-chunks per sub-gather
    gjw = gchunk // 16    # idx columns per sub-gather

    nc = bacc.Bacc("TRN2", target_bir_lowering=False, num_swdge_queues=4)
    coords = nc.dram_tensor("coords", [n_per_core, 3], F32, kind="ExternalInput")
    ptab = nc.dram_tensor("ptab", [3 * R, ES], F32, kind="ExternalInput")
    out = nc.dram_tensor("out", [n_per_core], F32, kind="ExternalOutput")

    with tile.TileContext(nc) as tc:
        with (
            tc.tile_pool(name="const", bufs=1) as cpool,
            tc.tile_pool(name="sb", bufs=bufs) as pool,
            tc.tile_pool(name="gt", bufs=bufs) as gpool,
            tc.tile_pool(name="dr", bufs=bufs, space="DRAM") as dpool,
        ):
            doffs = cpool.tile([128, 3 * nch], I16)
            for d in range(3):
                nc.vector.memset(doffs[:, d * nch : (d + 1) * nch], d * R)

            for t in range(tiles):
                cslice = coords.ap()[t * nt : (t + 1) * nt, :]

                # ---- coords + pos (block layout: partition p owns points
                # [p*nch, (p+1)*nch), laid out [128, (ch, xyz)]) ----
                cb = pool.tile([128, nch * 3], F32, tag="cb")
                nc.sync.dma_start(
                    cb[:, :], cslice.rearrange("(p j) c -> p (j c)", p=128))
                posb = pool.tile([128, nch * 3], F32, tag="posb")
                nc.scalar.activation(posb[:, :], cb[:, :], COPY, bias=0.5, scale=0.5)
                nc.scalar.activation(posb[:, :], posb[:, :], COPY, bias=0.0, scale=255.0)

                # ---- floor via cast + is_gt fixup (rounding-agnostic) ----
                r16 = pool.tile([128, nch * 3], I16, tag="r16")
                nc.vector.tensor_copy(r16[:, :], posb[:, :])
                rf = pool.tile([128, nch * 3], F32, tag="rf")
                nc.vector.tensor_copy(rf[:, :], r16[:, :])
                g = pool.tile([128, nch * 3], F32, tag="g")
                nc.vector.tensor_tensor(
                    out=g[:, :], in0=rf[:, :], in1=posb[:, :], op=ALU.is_gt)
                i0f = pool.tile([128, nch * 3], F32, tag="i0f")
                nc.vector.tensor_tensor(
                    out=i0f[:, :], in0=rf[:, :], in1=g[:, :], op=ALU.subtract)
                w = pool.tile([128, nch * 3], F32, tag="w")
                nc.vector.tensor_tensor(
                    out=w[:, :], in0=posb[:, :], in1=i0f[:, :], op=ALU.subtract)

                # ---- gather indices: pack per-dim, add 256*d, bounce via
                # DRAM to the wrapped (16-partition) + replicated layout ----
                idx16 = pool.tile([128, 3 * nch], I16, tag="idx16")
                nc.vector.tensor_copy(
                    idx16[:, :].rearrange("p (c j) -> p c j", c=3),
                    i0f[:, :].rearrange("p (j c) -> p c j", c=3))
                nc.vector.tensor_tensor(
                    out=idx16[:, :], in0=idx16[:, :], in1=doffs[:, :], op=ALU.add)
                dscr = dpool.tile([128, 3 * nch], I16, tag="dscr")
                nc.sync.dma_start(dscr[:, :], idx16[:, :])
                # gather slot i = ch*128 + (h*16+q) handles point
                # (h*16+q)*nch + ch; its idx sits at wrapped (q, s=ch*8+h),
                # replicated across the 8 groups of 16 partitions. One DMA
                # per h keeps both access patterns within 3 dims.
                ridx = pool.tile([128, 3 * jw], I16, tag="ridx")
                rv = ridx[:, :].rearrange("p (cch h) -> p cch h", h=8)
                for h in range(8):
                    nc.sync.dma_start(
                        rv[:, :, h],
                        dscr[h * 16 : (h + 1) * 16, :]
                        .unsqueeze(0).broadcast_to([8, 16, 3 * nch]))

                # ---- gathers (split: SWDGE rejects >~1k idxs/instruction) ----
                gts = []
                for d in range(3):
                    gt = gpool.tile([128, nch, ES], F32, tag=f"gt{d}")
                    for k in range(gsub):
                        nc.gpsimd.dma_gather(
                            gt[:, k * gnch : (k + 1) * gnch, :], ptab.ap(),
                            ridx[:, d * jw + k * gjw : d * jw + (k + 1) * gjw],
                            num_idxs=gchunk, num_idxs_reg=gchunk, elem_size=ES,
                            queue_num=(d * gsub + k) % 4)
                    gts.append(gt)

                # ---- interpolation + product + reduce ----
                tsc = pool.tile([128, nch, C], F32, tag="tsc")
                wv = w[:, :].rearrange("p (j c) -> p c j", c=3)
                for d in range(3):
                    wb = wv[:, d : d + 1, :].rearrange("p o j -> p (o j)") \
                        .unsqueeze(2).broadcast_to([128, nch, C])
                    nc.vector.tensor_tensor(
                        out=tsc[:, :, :], in0=gts[d][:, :, CP : CP + C],
                        in1=wb, op=ALU.mult)
                    nc.vector.tensor_tensor(
                        out=gts[d][:, :, 0:C], in0=tsc[:, :, :],
                        in1=gts[d][:, :, 0:C], op=ALU.add)
                m = pool.tile([128, nch, C], F32, tag="m")
                nc.vector.tensor_tensor(
                    out=m[:, :, :], in0=gts[0][:, :, 0:C], in1=gts[1][:, :, 0:C],
                    op=ALU.mult)
                nc.vector.tensor_tensor(
                    out=m[:, :, :], in0=m[:, :, :], in1=gts[2][:, :, 0:C],
                    op=ALU.mult)
                res = pool.tile([128, nch], F32, tag="res")
                nc.vector.tensor_reduce(
                    out=res[:, :], in_=m[:, :, :],
                    axis=mybir.AxisListType.X, op=ALU.add)
                nc.sync.dma_start(
                    out.ap()[t * nt : (t + 1) * nt].rearrange("(p j) -> p j", p=128),
                    res[:, :])
    nc.compile()
    return nc


_NC_CACHE = {}


def _get_nc():
    key = (N_PER_CORE, NT)
    if key not in _NC_CACHE:
        _NC_CACHE[key] = build_kernel()
    return _NC_CACHE[key]


def run(in_tensor: np.ndarray, line_coef: np.ndarray, trace: bool = False):
    """Returns (out [N_TOTAL] f32, BassKernelResults)."""
    from concourse.bass_utils import run_bass_kernel_spmd

    in_tensor = np.ascontiguousarray(in_tensor, dtype=np.float32)
    assert in_tensor.shape == (N_TOTAL, 3)
    ptab = build_ptab(np.asarray(line_coef))
    nc = _get_nc()
    shards = in_tensor.reshape(N_CORES, N_PER_CORE, 3)
    in_maps = [{"coords": shards[i], "ptab": ptab} for i in range(N_CORES)]
    res = run_bass_kernel_spmd(nc, in_maps, core_ids=list(range(N_CORES)),
                               trace=trace)
    out = np.concatenate([np.asarray(r["out"]) for r in res.results])
    return out, res


def kernel(in_tensor: np.ndarray, line_coef: np.ndarray) -> np.ndarray:
    out, _ = run(np.asarray(in_tensor), np.asarray(line_coef))
    return out



# revision 7
# speedup vs baseline: 9.7506x; 9.7506x over previous
"""TRN2 Bass kernel for CP-decoding line-sampling (nn_CPDecoding).

kernel(in_tensor [2097152,3] f32, line_coef [3,24,256] f32) -> [2097152] f32

Math per point n (reference semantics, align_corners grid_sample on R=256):
  pos_d = ((coord_d + 1) * 0.5) * 255          d=0,1,2 over (x,y,z) columns
  i0_d  = floor(pos_d); w_d = pos_d - i0_d
  f_d   = T_d[:, i0] + w_d * (T_d[:, i0+1] - T_d[:, i0])   (T_d = line_coef[2-d])
  out_n = sum_c f_0[c] * f_1[c] * f_2[c]

Strategy: data-parallel over points across 8 NeuronCores. Per core, SWDGE
dma_gather fetches one 256B pair-row (base row ++ delta row, 24->32 f32
padded) per (point, dim) from an HBM table [768, 64]; DVE computes
floor/frac (cast + is_gt fixup -- correct whether the f32->i16 cast rounds
or truncates), the interpolation, 3-way product and component-sum reduce.

The gather's wrapped+replicated idx layout (slot i = ch*128 + h*16 + q
handles point (h*16+q)*nch + ch; its idx sits at partition i%16 = q, col
i//16 = ch*8+h, replicated across the 8 groups of 16 partitions) is built
ON-CHIP: 8 partition-offset DVE copies assemble the [16, (d, ch, h)]
staging tile, and one SBUF->SBUF DMA replicates it to all 8 groups with
3KB-contiguous packets. (The previous DRAM-bounce produced 6.3M 2-byte
DMA packets = 17.2ms of the 19.96ms runtime.) Gathers are split into
1024-index chunks (the SWDGE ring rejects larger instructions here).
"""

import sys

try:
    import concourse.bass  # noqa: F401
except Exception:
    sys.path.insert(0, "/opt/trn_rl_repo")

import numpy as np

import concourse.bacc as bacc
import concourse.bass as bass
import concourse.mybir as mybir
import concourse.tile as tile

F32 = mybir.dt.float32
I16 = mybir.dt.int16
COPY = mybir.ActivationFunctionType.Copy
ALU = mybir.AluOpType

N_TOTAL = 2097152
N_CORES = 8
N_PER_CORE = N_TOTAL // N_CORES
R = 256
C = 24
CP = 32          # padded component stride
ES = 2 * CP      # gather elem_size (64 f32 = 256B)
NT = 8192        # points per tile


def build_ptab(line_coef: np.ndarray) -> np.ndarray:
    """[3, 24, 256] f32 -> [768, 64] pair table (base ++ delta, padded)."""
    line_coef = np.ascontiguousarray(line_coef, dtype=np.float32)
    assert line_coef.shape == (3, C, R)
    pt = np.zeros((3, R, ES), np.float32)
    for b in range(3):
        L = line_coef[2 - b]                      # [24, 256]
        pt[b, :, 0:C] = L.T
        pt[b, 0 : R - 1, CP : CP + C] = (L[:, 1:R] - L[:, 0 : R - 1]).T
    return pt.reshape(3 * R, ES)


def build_kernel(n_per_core: int = N_PER_CORE, nt: int = NT, bufs: int = 2,
                 gchunk: int = 1024):
    assert n_per_core % nt == 0 and nt % 2048 == 0
    assert nt % gchunk == 0 and gchunk % 128 == 0
    tiles = n_per_core // nt
    nch = nt // 128       # chunks (points per partition)
    jw = nt // 16         # wrapped idx columns per dim
    gsub = nt // gchunk   # sub-gathers per dim
    gnch = gchunk // 128  # point-chunks per sub-gather
    gjw = gchunk // 16    # idx columns per sub-gather

    nc = bacc.Bacc("TRN2", target_bir_lowering=False, num_swdge_queues=4)
    coords = nc.dram_tensor("coords", [n_per_core, 3], F32, kind="ExternalInput")
    ptab = nc.dram_tensor("ptab", [3 * R, ES], F32, kind="ExternalInput")
    out = nc.dram_tensor("out", [n_per_core], F32, kind="ExternalOutput")

    with tile.TileContext(nc) as tc:
        with (
            tc.tile_pool(name="const", bufs=1) as cpool,
            tc.tile_pool(name="sb", bufs=bufs) as pool,
            tc.tile_pool(name="gt", bufs=bufs) as gpool,
            tc.tile_pool(name="ps", bufs=bufs, space="PSUM") as ppool,
            tc.tile_pool(name="dr", bufs=bufs, space="DRAM") as dpool,
        ):
            # Per-dim table-base offsets (f32, added post-fold with the cast).
            doffsf = cpool.tile([128, 3], F32)
            for d in range(3):
                nc.vector.memset(doffsf[:, d : d + 1], float(d * R))
            # sel[p, j] = 1.0 iff p == j: identity used for PE partition folds
            # (lhsT = sel[:, h*16:(h+1)*16] extracts partitions h*16..+16).
            seli = cpool.tile([128, 128], mybir.dt.int32)
            nc.gpsimd.iota(seli[:, :], pattern=[[1, 128]], base=0,
                           channel_multiplier=0)
            selp = cpool.tile([128, 1], mybir.dt.int32)
            nc.gpsimd.iota(selp[:, :], pattern=[[0, 1]], base=0,
                           channel_multiplier=1)
            sel = cpool.tile([128, 128], F32)
            nc.vector.tensor_tensor(
                out=sel[:, :], in0=seli[:, :],
                in1=selp[:, :].broadcast_to([128, 128]), op=ALU.is_equal)

            for t in range(tiles):
                cslice = coords.ap()[t * nt : (t + 1) * nt, :]

                # ---- coords + pos (block layout: partition p owns points
                # [p*nch, (p+1)*nch), laid out [128, (ch, xyz)]) ----
                cb = pool.tile([128, nch * 3], F32, tag="cb")
                nc.sync.dma_start(
                    cb[:, :], cslice.rearrange("(p j) c -> p (j c)", p=128))
                posb = pool.tile([128, nch * 3], F32, tag="posb")
                nc.scalar.activation(posb[:, :], cb[:, :], COPY, bias=0.5, scale=0.5)
                nc.scalar.activation(posb[:, :], posb[:, :], COPY, bias=0.0, scale=255.0)

                # ---- floor via cast + is_gt fixup (rounding-agnostic) ----
                r16 = pool.tile([128, nch * 3], I16, tag="r16")
                nc.vector.tensor_copy(r16[:, :], posb[:, :])
                rf = pool.tile([128, nch * 3], F32, tag="rf")
                nc.vector.tensor_copy(rf[:, :], r16[:, :])
                g = pool.tile([128, nch * 3], F32, tag="g")
                nc.vector.tensor_tensor(
                    out=g[:, :], in0=rf[:, :], in1=posb[:, :], op=ALU.is_gt)
                i0f = pool.tile([128, nch * 3], F32, tag="i0f")
                nc.vector.tensor_tensor(
                    out=i0f[:, :], in0=rf[:, :], in1=g[:, :], op=ALU.subtract)
                w = pool.tile([128, nch * 3], F32, tag="w")
                nc.vector.tensor_tensor(
                    out=w[:, :], in0=posb[:, :], in1=i0f[:, :], op=ALU.subtract)

                # ---- wrapped idx layout, built on-chip ----
                # The gather wants slot i's idx at partition i%16; compute
                # lives at partition i%128.  Fold partitions h*16+q -> q with
                # PE selection matmuls (exact in f32; DVE can't read from
                # partition bases 16/48/80/112), then cast+interleave each
                # 16-row piece into stag[0:16, (d, ch, h)] columns on DVE.
                iv = i0f[:, :].rearrange("p (j c) -> p c j", c=3)
                stag = pool.tile([128, 3 * jw], I16, tag="stag")
                sv = stag[:, :].rearrange("p (d c h) -> p d c h", d=3, h=8)
                for h in range(8):
                    pfold = ppool.tile([16, 3 * nch], F32, tag="pfold")
                    pv = pfold[:, :].rearrange("p (d c) -> p d c", d=3)
                    for d in range(3):
                        nc.tensor.matmul(
                            pv[:, d, :], sel[:, h * 16 : (h + 1) * 16],
                            iv[:, d, :])
                    nc.vector.tensor_tensor(
                        out=sv[0:16, :, :, h], in0=pv[:, :, :],
                        in1=doffsf[0:16, :].unsqueeze(2)
                        .broadcast_to([16, 3, nch]), op=ALU.add)
                # Replicate [0:16] to all 8 groups of 16 partitions via a
                # small DRAM bounce: both sides move 3KB-contiguous runs
                # (SBUF-side partition broadcast isn't allowed, DRAM-side is).
                dscr = dpool.tile([16, 3 * jw], I16, tag="dscr")
                nc.sync.dma_start(dscr[:, :], stag[0:16, :])
                ridx = pool.tile([128, 3 * jw], I16, tag="ridx")
                nc.sync.dma_start(
                    ridx[:, :],
                    dscr[:, :].unsqueeze(0).broadcast_to([8, 16, 3 * jw]))

                # ---- gathers (split: SWDGE rejects >~1k idxs/instruction) ----
                gts = []
                for d in range(3):
                    gt = gpool.tile([128, nch, ES], F32, tag=f"gt{d}")
                    for k in range(gsub):
                        nc.gpsimd.dma_gather(
                            gt[:, k * gnch : (k + 1) * gnch, :], ptab.ap(),
                            ridx[:, d * jw + k * gjw : d * jw + (k + 1) * gjw],
                            num_idxs=gchunk, num_idxs_reg=gchunk, elem_size=ES,
                            queue_num=(d * gsub + k) % 4)
                    gts.append(gt)

                # ---- interpolation + product + reduce ----
                tsc = pool.tile([128, nch, C], F32, tag="tsc")
                wv = w[:, :].rearrange("p (j c) -> p c j", c=3)
                for d in range(3):
                    wb = wv[:, d : d + 1, :].rearrange("p o j -> p (o j)") \
                        .unsqueeze(2).broadcast_to([128, nch, C])
                    nc.vector.tensor_tensor(
                        out=tsc[:, :, :], in0=gts[d][:, :, CP : CP + C],
                        in1=wb, op=ALU.mult)
                    nc.vector.tensor_tensor(
                        out=gts[d][:, :, 0:C], in0=tsc[:, :, :],
                        in1=gts[d][:, :, 0:C], op=ALU.add)
                m = pool.tile([128, nch, C], F32, tag="m")
                nc.vector.tensor_tensor(
                    out=m[:, :, :], in0=gts[0][:, :, 0:C], in1=gts[1][:, :, 0:C],
                    op=ALU.mult)
                nc.vector.tensor_tensor(
                    out=m[:, :, :], in0=m[:, :, :], in1=gts[2][:, :, 0:C],
                    op=ALU.mult)
                res = pool.tile([128, nch], F32, tag="res")
                nc.vector.tensor_reduce(
                    out=res[:, :], in_=m[:, :, :],
                    axis=mybir.AxisListType.X, op=ALU.add)
                nc.sync.dma_start(
                    out.ap()[t * nt : (t + 1) * nt].rearrange("(p j) -> p j", p=128),
                    res[:, :])
    nc.compile()
    return nc


_NC_CACHE = {}


def _get_nc():
    key = (N_PER_CORE, NT)
    if key not in _NC_CACHE:
        _NC_CACHE[key] = build_kernel()
    return _NC_CACHE[key]


def run(in_tensor: np.ndarray, line_coef: np.ndarray, trace: bool = False):
    """Returns (out [N_TOTAL] f32, BassKernelResults)."""
    from concourse.bass_utils import run_bass_kernel_spmd

    in_tensor = np.ascontiguousarray(in_tensor, dtype=np.float32)
    assert in_tensor.shape == (N_TOTAL, 3)
    ptab = build_ptab(np.asarray(line_coef))
    nc = _get_nc()
    shards = in_tensor.reshape(N_CORES, N_PER_CORE, 3)
    in_maps = [{"coords": shards[i], "ptab": ptab} for i in range(N_CORES)]
    res = run_bass_kernel_spmd(nc, in_maps, core_ids=list(range(N_CORES)),
                               trace=trace)
    out = np.concatenate([np.asarray(r["out"]) for r in res.results])
    return out, res


def kernel(in_tensor: np.ndarray, line_coef: np.ndarray) -> np.ndarray:
    out, _ = run(np.asarray(in_tensor), np.asarray(line_coef))
    return out


# revision 10
# speedup vs baseline: 14.5762x; 1.4949x over previous
"""TRN2 Bass kernel for CP-decoding line-sampling (nn_CPDecoding).

kernel(in_tensor [2097152,3] f32, line_coef [3,24,256] f32) -> [2097152] f32

Math per point n (reference semantics, align_corners grid_sample on R=256):
  pos_d = ((coord_d + 1) * 0.5) * 255          d=0,1,2 over (x,y,z) columns
  i0_d  = floor(pos_d); w_d = pos_d - i0_d
  f_d   = T_d[:, i0] + w_d * (T_d[:, i0+1] - T_d[:, i0])   (T_d = line_coef[2-d])
  out_n = sum_c f_0[c] * f_1[c] * f_2[c]

Strategy: data-parallel over points across 8 NeuronCores. Per core, SWDGE
dma_gather fetches one 256B pair-row (base row ++ delta row, 24->32 f32
padded) per (point, dim) from an HBM table [768, 64]; DVE computes
floor/frac (cast + is_gt fixup -- correct whether the f32->i16 cast rounds
or truncates), the interpolation, 3-way product and component-sum reduce.

The gather's wrapped+replicated idx layout (slot i = ch*128 + h*16 + q
handles point (h*16+q)*nch + ch; its idx sits at partition i%16 = q, col
i//16 = ch*8+h, replicated across the 8 groups of 16 partitions) is built
ON-CHIP: 8 partition-offset DVE copies assemble the [16, (d, ch, h)]
staging tile, and one SBUF->SBUF DMA replicates it to all 8 groups with
3KB-contiguous packets. (The previous DRAM-bounce produced 6.3M 2-byte
DMA packets = 17.2ms of the 19.96ms runtime.) Gathers are split into
1024-index chunks (the SWDGE ring rejects larger instructions here).
"""

import sys

try:
    import concourse.bass  # noqa: F401
except Exception:
    sys.path.insert(0, "/opt/trn_rl_repo")

import numpy as np

import concourse.bacc as bacc
import concourse.bass as bass
import concourse.mybir as mybir
import concourse.tile as tile

F32 = mybir.dt.float32
I16 = mybir.dt.int16
COPY = mybir.ActivationFunctionType.Copy
ALU = mybir.AluOpType

N_TOTAL = 2097152
N_CORES = 8
N_PER_CORE = N_TOTAL // N_CORES
R = 256
C = 24
CP = 32          # padded component stride
ES = 2 * CP      # gather elem_size (64 f32 = 256B)
NT = 8192        # points per tile


def build_ptab(line_coef: np.ndarray) -> np.ndarray:
    """[3, 24, 256] f32 -> [768, 64] pair table (base ++ delta, padded)."""
    line_coef = np.ascontiguousarray(line_coef, dtype=np.float32)
    assert line_coef.shape == (3, C, R)
    pt = np.zeros((3, R, ES), np.float32)
    for b in range(3):
        L = line_coef[2 - b]                      # [24, 256]
        pt[b, :, 0:C] = L.T
        pt[b, 0 : R - 1, CP : CP + C] = (L[:, 1:R] - L[:, 0 : R - 1]).T
    return pt.reshape(3 * R, ES)


def build_kernel(n_per_core: int = N_PER_CORE, nt: int = NT, bufs: int = 2,
                 gchunk: int = 1024):
    assert n_per_core % nt == 0 and nt % 2048 == 0
    assert nt % gchunk == 0 and gchunk % 128 == 0
    tiles = n_per_core // nt
    nch = nt // 128       # chunks (points per partition)
    jw = nt // 16         # wrapped idx columns per dim
    gsub = nt // gchunk   # sub-gathers per dim
    gnch = gchunk // 128  # point-chunks per sub-gather
    gjw = gchunk // 16    # idx columns per sub-gather

    nc = bacc.Bacc("TRN2", target_bir_lowering=False, num_swdge_queues=4)
    coords = nc.dram_tensor("coords", [n_per_core, 3], F32, kind="ExternalInput")
    ptab = nc.dram_tensor("ptab", [3 * R, ES], F32, kind="ExternalInput")
    out = nc.dram_tensor("out", [n_per_core], F32, kind="ExternalOutput")

    with tile.TileContext(nc) as tc:
        with (
            tc.tile_pool(name="const", bufs=1) as cpool,
            tc.tile_pool(name="sb", bufs=bufs) as pool,
            tc.tile_pool(name="gt", bufs=bufs) as gpool,
            tc.tile_pool(name="ps", bufs=bufs, space="PSUM") as ppool,
            tc.tile_pool(name="dr", bufs=bufs, space="DRAM") as dpool,
        ):
            # Per-dim table-base offsets (f32, added post-fold with the cast).
            doffsf = cpool.tile([128, 3], F32)
            for d in range(3):
                nc.vector.memset(doffsf[:, d : d + 1], float(d * R))
            # sel[p, j] = 1.0 iff p == j: identity used for PE partition folds
            # (lhsT = sel[:, h*16:(h+1)*16] extracts partitions h*16..+16).
            seli = cpool.tile([128, 128], mybir.dt.int32)
            nc.gpsimd.iota(seli[:, :], pattern=[[1, 128]], base=0,
                           channel_multiplier=0)
            selp = cpool.tile([128, 1], mybir.dt.int32)
            nc.gpsimd.iota(selp[:, :], pattern=[[0, 1]], base=0,
                           channel_multiplier=1)
            sel = cpool.tile([128, 128], F32)
            nc.vector.tensor_tensor(
                out=sel[:, :], in0=seli[:, :],
                in1=selp[:, :].broadcast_to([128, 128]), op=ALU.is_equal)

            for t in range(tiles):
                cslice = coords.ap()[t * nt : (t + 1) * nt, :]

                # ---- coords + pos (block layout: partition p owns points
                # [p*nch, (p+1)*nch), laid out [128, (ch, xyz)]) ----
                cb = pool.tile([128, nch * 3], F32, tag="cb")
                nc.sync.dma_start(
                    cb[:, :], cslice.rearrange("(p j) c -> p (j c)", p=128))
                posb = pool.tile([128, nch * 3], F32, tag="posb")
                nc.scalar.activation(posb[:, :], cb[:, :], COPY, bias=0.5, scale=0.5)
                nc.scalar.activation(posb[:, :], posb[:, :], COPY, bias=0.0, scale=255.0)

                # ---- floor via cast + is_gt fixup (rounding-agnostic) ----
                r16 = pool.tile([128, nch * 3], I16, tag="r16")
                nc.vector.tensor_copy(r16[:, :], posb[:, :])
                rf = pool.tile([128, nch * 3], F32, tag="rf")
                nc.vector.tensor_copy(rf[:, :], r16[:, :])
                g = pool.tile([128, nch * 3], F32, tag="g")
                nc.vector.tensor_tensor(
                    out=g[:, :], in0=rf[:, :], in1=posb[:, :], op=ALU.is_gt)
                i0f = pool.tile([128, nch * 3], F32, tag="i0f")
                nc.vector.tensor_tensor(
                    out=i0f[:, :], in0=rf[:, :], in1=g[:, :], op=ALU.subtract)
                w = pool.tile([128, nch * 3], F32, tag="w")
                nc.vector.tensor_tensor(
                    out=w[:, :], in0=posb[:, :], in1=i0f[:, :], op=ALU.subtract)

                # ---- wrapped idx layout, built on-chip ----
                # The gather wants slot i's idx at partition i%16; compute
                # lives at partition i%128.  Fold partitions h*16+q -> q with
                # PE selection matmuls (exact in f32; DVE can't read from
                # partition bases 16/48/80/112), then cast+interleave each
                # 16-row piece into stag[0:16, (d, ch, h)] columns on DVE.
                iv = i0f[:, :].rearrange("p (j c) -> p c j", c=3)
                stag = pool.tile([128, 3 * jw], I16, tag="stag")
                sv = stag[:, :].rearrange("p (d c h) -> p d c h", d=3, h=8)
                for h in range(8):
                    pfold = ppool.tile([16, 3 * nch], F32, tag="pfold")
                    pv = pfold[:, :].rearrange("p (d c) -> p d c", d=3)
                    for d in range(3):
                        nc.tensor.matmul(
                            pv[:, d, :], sel[:, h * 16 : (h + 1) * 16],
                            iv[:, d, :])
                    nc.vector.tensor_tensor(
                        out=sv[0:16, :, :, h], in0=pv[:, :, :],
                        in1=doffsf[0:16, :].unsqueeze(2)
                        .broadcast_to([16, 3, nch]), op=ALU.add)
                # Replicate [0:16] to all 8 groups of 16 partitions via a
                # small DRAM bounce: both sides move 3KB-contiguous runs
                # (SBUF-side partition broadcast isn't allowed, DRAM-side is).
                dscr = dpool.tile([16, 3 * jw], I16, tag="dscr")
                nc.sync.dma_start(dscr[:, :], stag[0:16, :])
                ridx = pool.tile([128, 3 * jw], I16, tag="ridx")
                nc.sync.dma_start(
                    ridx[:, :],
                    dscr[:, :].unsqueeze(0).broadcast_to([8, 16, 3 * jw]))

                # ---- gathers (split: SWDGE rejects >~1k idxs/instruction) ----
                gts = []
                for d in range(3):
                    gt = gpool.tile([128, nch, ES], F32, tag=f"gt{d}")
                    for k in range(gsub):
                        nc.gpsimd.dma_gather(
                            gt[:, k * gnch : (k + 1) * gnch, :], ptab.ap(),
                            ridx[:, d * jw + k * gjw : d * jw + (k + 1) * gjw],
                            num_idxs=gchunk, num_idxs_reg=gchunk, elem_size=ES,
                            queue_num=(d * gsub + k) % 4)
                    gts.append(gt)

                # ---- interpolation + product + reduce ----
                tsc = pool.tile([128, nch, C], F32, tag="tsc")
                wv = w[:, :].rearrange("p (j c) -> p c j", c=3)
                for d in range(3):
                    wb = wv[:, d : d + 1, :].rearrange("p o j -> p (o j)") \
                        .unsqueeze(2).broadcast_to([128, nch, C])
                    nc.vector.tensor_tensor(
                        out=tsc[:, :, :], in0=gts[d][:, :, CP : CP + C],
                        in1=wb, op=ALU.mult)
                    nc.vector.tensor_tensor(
                        out=gts[d][:, :, 0:C], in0=tsc[:, :, :],
                        in1=gts[d][:, :, 0:C], op=ALU.add)
                m = pool.tile([128, nch, C], F32, tag="m")
                nc.vector.tensor_tensor(
                    out=m[:, :, :], in0=gts[0][:, :, 0:C], in1=gts[1][:, :, 0:C],
                    op=ALU.mult)
                nc.vector.tensor_tensor(
                    out=m[:, :, :], in0=m[:, :, :], in1=gts[2][:, :, 0:C],
                    op=ALU.mult)
                res = pool.tile([128, nch], F32, tag="res")
                nc.vector.tensor_reduce(
                    out=res[:, :], in_=m[:, :, :],
                    axis=mybir.AxisListType.X, op=ALU.add)
                nc.sync.dma_start(
                    out.ap()[t * nt : (t + 1) * nt].rearrange("(p j) -> p j", p=128),
                    res[:, :])
    nc.compile()
    return nc


_NC_CACHE = {}


def _get_nc():
    key = (N_PER_CORE, NT)
    if key not in _NC_CACHE:
        _NC_CACHE[key] = build_kernel()
    return _NC_CACHE[key]


def run(in_tensor: np.ndarray, line_coef: np.ndarray, trace: bool = False):
    """Returns (out [N_TOTAL] f32, BassKernelResults)."""
    from concourse.bass_utils import run_bass_kernel_spmd

    in_tensor = np.ascontiguousarray(in_tensor, dtype=np.float32)
    assert in_tensor.shape == (N_TOTAL, 3)
    ptab = build_ptab(np.asarray(line_coef))
    nc = _get_nc()
    shards = in_tensor.reshape(N_CORES, N_PER_CORE, 3)
    in_maps = [{"coords": shards[i], "ptab": ptab} for i in range(N_CORES)]
    res = run_bass_kernel_spmd(nc, in_maps, core_ids=list(range(N_CORES)),
                               trace=trace)
    out = np.concatenate([np.asarray(r["out"]) for r in res.results])
    return out, res


def kernel(in_tensor: np.ndarray, line_coef: np.ndarray) -> np.ndarray:
    out, _ = run(np.asarray(in_tensor), np.asarray(line_coef))
    return out
